# revision 9
# baseline (speedup 1.0000x reference)
"""Trainium2 Bass kernel for CausalPrefixMemory.

Computes, for x [B, S, D], W_update/W_gate [SD, D], W_out [D, SD]:
    gate = sigmoid(x @ W_gate.T); upd = x @ W_update.T
    memory = cumsum(gate * upd, axis=1)            # [B, S, SD]
    h = memory / arange(1, S+1)[None, :, None]
    h = h * rsqrt(mean(h*h, -1) + eps)             # RMSNorm, no weight
    out = h @ W_out.T                              # [B, S, D]

Sharding over 8 NeuronCores: (batch, sequence-half).  Core c < 4 handles
rows [0, S/2) of batch c; core c+4 handles rows [S/2, S) of batch c.  The
only cross-core dependency is the running prefix total of the first half
(a [SD] vector), exchanged with a 2-rank AllReduce per batch pair.

Within a core the sequence is processed in 128-row chunks.  x arrives
host-pre-transposed (bf16, chunk-tiled) so the projections need no PE
transposes.  The cumsum runs in TRANSPOSED state layout: for each chunk,
memT[k, t] = sum_{s<=t} gu[s, k] is 4 [128]x[128,128] matmuls against an
upper-triangular ones matrix; the cross-chunk carry is then a [128,4]
per-partition vector add (free on the PE), the carry chain itself reads
memT[:, :, 127] slices, and memT is directly the stationary operand of
the output projection, so no h transposes are needed either.  The RMS
row-sums come from 4 single-column matmuls on the squared tiles; the
1/position and rsqrt scales are folded into the PSUM->SBUF copy of the
output rows on the Act engine.

All heavy matmuls are bf16 (full PE stream rate); the carry chain stays
fp32.  Per 128-row chunk the PE executes ~12.8K cycles vs the 16.4K of
the previous version; both phases are software-pipelined by one chunk so
the PE never waits on the Act/DVE chain.
"""

import sys

import numpy as np

if "/opt/trn_rl_repo" not in sys.path:
    sys.path.insert(0, "/opt/trn_rl_repo")

import concourse.bass as bass  # noqa: E402
import concourse.tile as tile  # noqa: E402
from concourse import bacc, mybir  # noqa: E402
from concourse.bass_utils import run_bass_kernel_spmd  # noqa: E402

F32 = mybir.dt.float32
BF16 = mybir.dt.bfloat16

B, S, D, SD = 4, 8192, 1024, 512
N_CORES = 8
P = 128  # chunk rows == SBUF partitions
ND = D // P  # 8 d-blocks
NK = SD // P  # 4 k-blocks
RMS_EPS = 1.1920929e-07

TRACE = False
LAST_EXEC_NS = None

AF = mybir.ActivationFunctionType


def emit_core_kernel(tc, io, n_chunks, pair_groups):
    """Emit the per-core program. io maps names -> DRAM APs."""
    nc = tc.nc
    xT = io["xT"]          # [n_chunks*P, ND*P] bf16, chunk c rows c*P..c*P+P
    out = io["out"]        # [n_chunks*P, D] bf16

    with (
        tc.tile_pool(name="statics", bufs=1) as statics,
        tc.tile_pool(name="gu_pool", bufs=1) as gu_pool,
        tc.tile_pool(name="xin", bufs=3) as xin,
        tc.tile_pool(name="work", bufs=2) as work,
        tc.tile_pool(name="hbuf", bufs=3) as hbuf,
        tc.tile_pool(name="obuf", bufs=3) as obuf,
        tc.tile_pool(name="small", bufs=4) as small,
        tc.tile_pool(name="dram", bufs=1, space="DRAM") as dram,
    ):
        # ---- static tiles: small tables first, weights in consumption
        # order so chunk 0 isn't blocked on later slabs ----
        utri_sb = statics.tile([P, P], BF16)
        nc.sync.dma_start(out=utri_sb, in_=io["utri"])
        onec_sb = statics.tile([P, 1], BF16)
        nc.sync.dma_start(out=onec_sb, in_=io["onec"])
        recip_sb = statics.tile([P, n_chunks], F32)
        nc.sync.dma_start(out=recip_sb, in_=io["recip"])
        r2d_sb = statics.tile([P, n_chunks], F32)
        nc.sync.dma_start(out=r2d_sb, in_=io["r2d"])
        msend_sb = statics.tile([P, 1], F32)
        nc.sync.dma_start(out=msend_sb, in_=io["mask_send"])
        mrecv_sb = statics.tile([P, 1], F32)
        nc.sync.dma_start(out=mrecv_sb, in_=io["mask_recv"])
        wg_sb = statics.tile([P, ND, SD], BF16)
        wu_sb = statics.tile([P, ND, SD], BF16)
        wgT_r = io["wgT"].rearrange("(j p) k -> p j k", p=P)
        wuT_r = io["wuT"].rearrange("(j p) k -> p j k", p=P)
        for j in range(ND):
            nc.sync.dma_start(out=wg_sb[:, j, :], in_=wgT_r[:, j, :])
            nc.sync.dma_start(out=wu_sb[:, j, :], in_=wuT_r[:, j, :])
        wo_sb = statics.tile([P, NK, D], BF16)
        nc.sync.dma_start(out=wo_sb, in_=io["woT"].rearrange("(j p) d -> p j d", p=P))

        zero_sb = statics.tile([P, 1], F32)
        nc.vector.memset(zero_sb, 0.0)
        eps_sb = statics.tile([P, 1], F32)
        nc.vector.memset(eps_sb, RMS_EPS)

        gu_tiles = []

        # ---- phase 1: projections + gu + transposed running total --------
        with (
            tc.tile_pool(name="ps_gate", bufs=2, space="PSUM") as ps_gate,
            tc.tile_pool(name="ps_upd", bufs=2, space="PSUM") as ps_upd,
            tc.tile_pool(name="ps_tot", bufs=1, space="PSUM") as ps_tot,
        ):
            # running total over all chunks, in transposed [k, kb] layout;
            # 4 single-column accumulation groups spanning the whole phase
            # (separate tiles: one pending group per PSUM zero region)
            totT_ps = [
                ps_tot.tile([P, 1], F32, tag=f"tot{kb}", name=f"tot{kb}")
                for kb in range(NK)
            ]

            def emit_tot(c):
                g = gu_tiles[c]
                for kb in range(NK):
                    nc.tensor.matmul(
                        totT_ps[kb],
                        g[:, kb * P : (kb + 1) * P],
                        onec_sb,
                        start=(c == 0),
                        stop=(c == n_chunks - 1),
                    )

            for c in range(n_chunks):
                x_sb = xin.tile([P, ND * P], BF16, tag="x", name=f"x{c}")
                nc.gpsimd.dma_start(out=x_sb, in_=xT[c * P : (c + 1) * P, :])

                gate_ps = ps_gate.tile([P, SD], F32, tag="g", name=f"g{c}")
                upd_ps = ps_upd.tile([P, SD], F32, tag="u", name=f"u{c}")
                for j in range(ND):
                    lhsT = x_sb[:, j * P : (j + 1) * P]
                    nc.tensor.matmul(
                        gate_ps, lhsT, wg_sb[:, j, :],
                        start=(j == 0), stop=(j == ND - 1),
                    )
                    nc.tensor.matmul(
                        upd_ps, lhsT, wu_sb[:, j, :],
                        start=(j == 0), stop=(j == ND - 1),
                    )
                # software pipeline: the previous chunk's total (which waits
                # on Act+DVE) goes after this chunk's projections
                if c >= 1:
                    emit_tot(c - 1)

                sig_sb = work.tile([P, SD], F32, tag="sig", name=f"sig{c}")
                nc.scalar.activation(
                    sig_sb, gate_ps, AF.Sigmoid, bias=zero_sb,
                )
                gu_sb = gu_pool.tile([P, SD], BF16, tag=f"gu{c}", name=f"gu{c}")
                nc.vector.tensor_mul(gu_sb, sig_sb, upd_ps)
                gu_tiles.append(gu_sb)
            emit_tot(n_chunks - 1)

            # masked carry exchange: first-half cores contribute their
            # total, second-half cores receive it
            contrib_sb = small.tile([P, NK], F32, tag="contrib", bufs=1)
            for kb in range(NK):
                nc.vector.tensor_scalar_mul(
                    contrib_sb[:, kb : kb + 1], totT_ps[kb], msend_sb
                )
            # AllGather (cheaper than AllReduce) + on-core add of the two
            # rank slices; the send-side mask keeps the sum == pair total
            cc_in = dram.tile([P, NK], F32, tag="cc_in")
            cc_out = dram.tile([2 * P, NK], F32, tag="cc_out")
            nc.sync.dma_start(out=cc_in, in_=contrib_sb)
            nc.gpsimd.collective_compute(
                "AllGather",
                mybir.AluOpType.bypass,
                replica_groups=pair_groups,
                ins=[cc_in.opt()],
                outs=[cc_out.opt()],
            )
            rraw_sb = small.tile([P, 2, NK], F32, tag="rraw", bufs=1)
            nc.sync.dma_start(
                out=rraw_sb, in_=cc_out.rearrange("(r p) k -> p r k", p=P)
            )
            rsum_sb = small.tile([P, NK], F32, tag="rsum", bufs=1)
            nc.vector.tensor_add(rsum_sb, rraw_sb[:, 0, :], rraw_sb[:, 1, :])
            r_sb = small.tile([P, NK], F32, tag="rrow", bufs=1)
            nc.vector.tensor_scalar_mul(r_sb, rsum_sb, mrecv_sb)

        # ---- phase 2: transposed cumsum + carry + RMS + output proj -------
        with (
            tc.tile_pool(name="ps_mem", bufs=2, space="PSUM") as ps_mem,
            tc.tile_pool(name="ps_msum", bufs=2, space="PSUM") as ps_msum,
            tc.tile_pool(name="ps_out", bufs=3, space="PSUM") as ps_out,
        ):
            def emit_cumsum(c):
                memT_ps = ps_mem.tile([P, NK, P], F32, tag="mem", name=f"mem{c}")
                g = gu_tiles[c]
                for kb in range(NK):
                    nc.tensor.matmul(
                        memT_ps[:, kb, :],
                        g[:, kb * P : (kb + 1) * P],
                        utri_sb,
                        start=True, stop=True,
                    )
                return memT_ps

            def emit_carry(c, memT_ps, carry):
                # memT_sb = memT_ps + carry  (per-partition broadcast along t)
                memT_sb = hbuf.tile([P, NK, P], BF16, tag="h", name=f"h{c}")
                for kb in range(NK):
                    nc.vector.tensor_scalar_add(
                        memT_sb[:, kb, :], memT_ps[:, kb, :], carry[:, kb : kb + 1]
                    )
                carry_next = small.tile(
                    [P, NK], F32, tag="carry", name=f"carry{c}", bufs=3
                )
                nc.vector.tensor_add(carry_next, carry, memT_ps[:, :, P - 1])
                sq_sb = work.tile([P, NK, P], BF16, tag="sq", name=f"sq{c}")
                nc.vector.tensor_mul(sq_sb, memT_sb, memT_sb)
                return memT_sb, sq_sb, carry_next

            carry = r_sb
            memT_ps = emit_cumsum(0)
            memT_sb, sq_sb, carry = emit_carry(0, memT_ps, carry)

            for c in range(n_chunks):
                if c + 1 < n_chunks:
                    memT_ps_n = emit_cumsum(c + 1)
                    memT_sb_n, sq_sb_n, carry = emit_carry(c + 1, memT_ps_n, carry)

                # msum[t] = sum_k memT_sb[k, t]^2 via 4 single-column matmuls
                msum_ps = ps_msum.tile([P, 1], F32, tag="msum", name=f"msum{c}")
                for kb in range(NK):
                    nc.tensor.matmul(
                        msum_ps, sq_sb[:, kb, :], onec_sb,
                        start=(kb == 0), stop=(kb == NK - 1),
                    )
                # scale = recip * rsqrt(msum*recip^2/SD + eps)
                sqt_sb = small.tile([P, 1], F32, tag="sqt", name=f"sqt{c}")
                nc.scalar.activation(
                    sqt_sb, msum_ps, AF.Sqrt,
                    bias=eps_sb, scale=r2d_sb[:, c : c + 1],
                )
                rstd_sb = small.tile([P, 1], F32, tag="rstd", name=f"rstd{c}")
                nc.vector.reciprocal(rstd_sb, sqt_sb)
                scale_sb = small.tile([P, 1], F32, tag="scale", name=f"scale{c}")
                nc.vector.tensor_mul(scale_sb, rstd_sb, recip_sb[:, c : c + 1])

                out_sb = obuf.tile([P, D], BF16, tag="o", name=f"o{c}")
                for half in range(2):
                    op_ps = ps_out.tile(
                        [P, D // 2], F32, tag="op", name=f"op{c}_{half}"
                    )
                    for kb in range(NK):
                        nc.tensor.matmul(
                            op_ps,
                            memT_sb[:, kb, :],
                            wo_sb[:, kb, half * (D // 2) : (half + 1) * (D // 2)],
                            start=(kb == 0), stop=(kb == NK - 1),
                        )
                    nc.scalar.activation(
                        out_sb[:, half * (D // 2) : (half + 1) * (D // 2)],
                        op_ps, AF.Copy, bias=0.0, scale=scale_sb,
                    )
                nc.sync.dma_start(out=out[c * P : (c + 1) * P, :], in_=out_sb)

                if c + 1 < n_chunks:
                    memT_ps, memT_sb, sq_sb = memT_ps_n, memT_sb_n, sq_sb_n


def aux_inputs(core, n_cores, s_local):
    """Per-core constant tables (host side)."""
    n_chunks = s_local // P
    first_half = core < n_cores // 2
    s0 = 0 if first_half else s_local
    pos = s0 + np.arange(n_chunks)[None, :] * P + np.arange(P)[:, None] + 1
    recip = (1.0 / pos).astype(np.float32)
    r2d = (recip * recip / np.float32(SD)).astype(np.float32)
    return {
        "recip": recip,
        "r2d": r2d,
        "mask_send": np.full((P, 1), 1.0 if first_half else 0.0, np.float32),
        "mask_recv": np.full((P, 1), 0.0 if first_half else 1.0, np.float32),
    }


def const_inputs(n_chunks):
    """Constant tables shared by all cores (host side)."""
    import ml_dtypes
    return {
        "utri": np.triu(np.ones((P, P), np.float32)).astype(ml_dtypes.bfloat16),
        "onec": np.ones((P, 1), np.float32).astype(ml_dtypes.bfloat16),
    }


_BUILD_CACHE = {}


def build(n_cores, s_local):
    key = (n_cores, s_local)
    if key in _BUILD_CACHE:
        return _BUILD_CACHE[key]
    n_chunks = s_local // P
    pair_groups = [[i, i + n_cores // 2] for i in range(n_cores // 2)]

    nc = bacc.Bacc(
        "TRN2", target_bir_lowering=False, debug=False, num_devices=n_cores
    )
    io = {}
    io["xT"] = nc.dram_tensor(
        "xT", [s_local, ND * P], BF16, kind="ExternalInput"
    ).ap()
    io["wgT"] = nc.dram_tensor("wgT", [D, SD], BF16, kind="ExternalInput").ap()
    io["wuT"] = nc.dram_tensor("wuT", [D, SD], BF16, kind="ExternalInput").ap()
    io["woT"] = nc.dram_tensor("woT", [SD, D], BF16, kind="ExternalInput").ap()
    for name, shape, dt_ in [
        ("utri", [P, P], BF16),
        ("onec", [P, 1], BF16),
        ("recip", [P, n_chunks], F32),
        ("r2d", [P, n_chunks], F32),
        ("mask_send", [P, 1], F32),
        ("mask_recv", [P, 1], F32),
    ]:
        io[name] = nc.dram_tensor(name, shape, dt_, kind="ExternalInput").ap()
    io["out"] = nc.dram_tensor(
        "out", [s_local, D], BF16, kind="ExternalOutput"
    ).ap()

    with tile.TileContext(nc) as tc:
        emit_core_kernel(tc, io, n_chunks, pair_groups)
    nc.compile()
    _BUILD_CACHE[key] = nc
    return nc


def prepare(x, W_update, W_gate, W_out):
    """Host-side prep: build the module and the per-core input maps."""
    import ml_dtypes
    x = np.asarray(x, np.float32)
    wgT = np.ascontiguousarray(
        np.asarray(W_gate, np.float32).T.astype(ml_dtypes.bfloat16)
    )
    wuT = np.ascontiguousarray(
        np.asarray(W_update, np.float32).T.astype(ml_dtypes.bfloat16)
    )
    woT = np.ascontiguousarray(
        np.asarray(W_out, np.float32).T.astype(ml_dtypes.bfloat16)
    )

    s_local = S // 2
    n_chunks = s_local // P
    nc = build(N_CORES, s_local)

    consts = const_inputs(n_chunks)
    # chunk-tiled transposed x: [c, t, j, p] -> [c, p, j, t], flattened 2D
    xs = (
        x.reshape(B, 2, n_chunks, P, ND, P)
        .transpose(0, 1, 2, 5, 4, 3)
        .astype(ml_dtypes.bfloat16)
    )
    in_maps = []
    for core in range(N_CORES):
        b, half = core % B, core // B  # cores 0-3 first halves, 4-7 second
        m = {
            "xT": np.ascontiguousarray(xs[b, half]).reshape(s_local, ND * P),
            "wgT": wgT,
            "wuT": wuT,
            "woT": woT,
            **consts,
            **aux_inputs(core, N_CORES, s_local),
        }
        in_maps.append(m)
    return nc, in_maps


def unshard(results):
    """Gather per-core 'out' arrays back to the full [B, S, D] output."""
    s_local = S // 2
    out = np.empty((B, 2, s_local, D), np.float32)
    for core in range(N_CORES):
        b, half = core % B, core // B
        out[b, half] = np.asarray(results[core]["out"]).astype(np.float32)
    return out.reshape(B, S, D)


def kernel(x, W_update, W_gate, W_out):
    global LAST_EXEC_NS
    nc, in_maps = prepare(x, W_update, W_gate, W_out)

    try:
        res = run_bass_kernel_spmd(
            nc, in_maps, core_ids=list(range(N_CORES)), trace=TRACE
        )
    except ModuleNotFoundError:
        # NTFF profile hook unavailable in this environment
        res = run_bass_kernel_spmd(
            nc, in_maps, core_ids=list(range(N_CORES)), trace=False
        )
    LAST_EXEC_NS = res.exec_time_ns
    return unshard(res.results)
